# revision 1
# baseline (speedup 1.0000x reference)
"""Trainium2 Bass kernel for an 8-expert top-2 MoE layer.

Strategy (expert-parallel, per the sharding hint "all-to-all tokens by
top-k assignment"): the host computes the (tiny) gating matmul + softmax
+ top-2 routing, gathers each expert's assigned tokens, and ships one
expert per NeuronCore. Each core runs the heavy 2-layer MLP for its
expert over its assigned tokens (f32r matmuls on the PE array), applies
the gate weights on-device, and the host scatter-adds the two expert
contributions per token back together.

The MLP math runs fully transposed (tokens on the free dim) so that
 - W1/W2 slices feed the PE as stationary operands with no transposes,
 - the b1 bias + relu and (y + b2) * gate evictions are single fused
   DVE ops with per-partition scalars,
 - the per-token gate row is broadcast across partitions with one K=1
   matmul (ones[1,128]^T @ g[1,N] -> G[128,N]).

DMA-dispatch overhead (~0.6us per dma_start, serialized on the DGE
queue) is first-order here, so transfers are batched: weights are
shipped as eight j-strip (o-strip) tensors host-packed to [128, 8, 128]
so each strip is one DMA and gates exactly one accumulation group, x
arrives as one DMA per token tile (host-packed [128, 8, C]), and the
biases/gates land in one merged DMA each. Token tiles are 512 wide
(one fp32 PSUM bank) with a final 256-wide tile to trim padding (f32r
keeps full rate at free dim >= 256). A few dummy matmuls run in the
shadow of the initial DMA ramp to engage the PE clock-gate (HAM)
before the real matmuls arrive.
"""

import numpy as np

NUM_EXPERTS = 8
TOP_K = 2
D = 1024

_prog_cache = {}


def _plan_tiles(max_load):
    """Token-tile sizes covering max_load: 512s with a final 256 when it fits."""
    n256 = -(-max_load // 256)
    tiles = [512] * (n256 // 2)
    if n256 % 2 == 1:
        tiles.append(256)
    if not tiles:
        tiles = [256]
    return sum(tiles), tiles


def _build_program(tile_plan):
    """Build the per-core Bass program: one expert's MLP over C tokens."""
    from contextlib import ExitStack

    import concourse.tile as tile
    from concourse import bacc, mybir

    f32 = mybir.dt.float32
    f32r = mybir.dt.float32r
    ADD = mybir.AluOpType.add
    MAX = mybir.AluOpType.max
    MULT = mybir.AluOpType.mult

    C, tok_tiles = tile_plan

    nc = bacc.Bacc("TRN2", target_bir_lowering=False, debug=False,
                   num_devices=NUM_EXPERTS)

    # host-packed layouts (see _make_in_maps):
    #   xT:  [128, 8, C]      xT[p, d, c] = x_gathered[c, d*128+p]
    #   w1:  [8, 128, 8, 128] w1[j, p, d, r] = W1[d*128+p, j*128+r]
    #   w2:  [8, 128, 8, 128] w2[o, p, j, r] = W2[j*128+p, o*128+r]
    #   b1:  [128, 8]         b1[p, j] = b1[j*128+p]   (b2 same)
    #   yT:  [128, 8, C]      yT[p, o, c] = y[c, o*128+p] * gate[c]
    xT_d = nc.dram_tensor("xT", [128, 8, C], f32r, kind="ExternalInput").ap()
    w1_d = nc.dram_tensor("w1", [8, 128, 8, 128], f32r, kind="ExternalInput").ap()
    w2_d = nc.dram_tensor("w2", [8, 128, 8, 128], f32r, kind="ExternalInput").ap()
    bb_d = nc.dram_tensor("bb", [128, 16], f32, kind="ExternalInput").ap()
    go_d = nc.dram_tensor("go", [1, C + 128], f32r, kind="ExternalInput").ap()
    yT_d = nc.dram_tensor("yT", [128, 8, C], f32, kind="ExternalOutput").ap()

    with tile.TileContext(nc) as tc, ExitStack() as ctx:
        wpool = ctx.enter_context(tc.tile_pool(name="w", bufs=1))
        cpool = ctx.enter_context(tc.tile_pool(name="const", bufs=1))
        xpool = ctx.enter_context(tc.tile_pool(name="x", bufs=2))
        hpool = ctx.enter_context(tc.tile_pool(name="h", bufs=2))
        ypool = ctx.enter_context(tc.tile_pool(name="y", bufs=2))
        gpool = ctx.enter_context(tc.tile_pool(name="g", bufs=2))
        php = ctx.enter_context(tc.tile_pool(name="ph", bufs=3, space="PSUM"))
        pyp = ctx.enter_context(tc.tile_pool(name="py", bufs=3, space="PSUM"))
        pgp = ctx.enter_context(tc.tile_pool(name="pg", bufs=2, space="PSUM"))

        # tiny constants on the ACT DGE queue (parallel with the weight
        # stream on the SP queue), merged into single transfers:
        # bb = [b1 | b2] per-partition, go = [gate row | ones row]
        bb_sb = cpool.tile([128, 16], f32, tag="bb")
        nc.sync.dma_start(bb_sb[:], bb_d[:])
        b1_sb = bb_sb[:, 0:8]
        b2_sb = bb_sb[:, 8:16]
        go_sb = cpool.tile([1, C + 128], f32r, tag="go")
        nc.sync.dma_start(go_sb[:], go_d[:])
        g_sb = go_sb[:, 0:C]
        ones_sb = go_sb[:, C:C + 128]

        # PE warm-up in the shadow of the initial DMA ramp: ~4us of dummy
        # K=1 matmuls (gated only on the tiny g/ones transfers) keep the
        # HAM activity monitor busy so the real matmuls run at 2.4 GHz
        warm = pgp.tile([128, min(C, 512)], f32, tag="gps")
        for _ in range(6):
            nc.tensor.matmul(warm[:], ones_sb[:], g_sb[:, 0:min(C, 512)],
                             start=True, stop=True)

        # DMA emission in consumption order: w1 strip 0, then the first
        # token tile of x^T per d-block (the j=0 group's d-MMs start as each
        # block lands), then the remaining w1 strips (one gates each j-group)
        TT0 = tok_tiles[0]
        w1_sb = [None] * 8
        w1_first = wpool.tile([128, 8 * 128], f32r, tag="w1_0")
        nc.sync.dma_start(w1_first[:], w1_d[0])
        w1_sb[0] = w1_first
        x0a = xpool.tile([128, 4 * TT0], f32r, tag="x0a")
        nc.sync.dma_start(x0a[:], xT_d[:, 0:4, 0:TT0])
        x0b = xpool.tile([128, 4 * TT0], f32r, tag="x0b")
        nc.sync.dma_start(x0b[:], xT_d[:, 4:8, 0:TT0])
        x_sb0 = (x0a, x0b)
        for j in range(1, 8):
            w1_strip = wpool.tile([128, 8 * 128], f32r, tag=f"w1_{j}")
            nc.sync.dma_start(w1_strip[:], w1_d[j])
            w1_sb[j] = w1_strip

        # w2 o-strips next: strip o gates tile 0's layer-2 o-group, which
        # starts right after layer 1 (~the w1 stream), so these must not
        # queue behind the second x tile; the second x tile (needed only
        # when tile 0 fully finishes) slots in before the last strip
        x_tiles = [None] * len(tok_tiles)
        x_tiles[0] = x_sb0
        w2_sb = [None] * 8
        for o in range(8):
            if o == 6 and len(tok_tiles) > 1:
                TT1 = tok_tiles[1]
                x1a = xpool.tile([128, 4 * TT1], f32r, tag="x0a")
                nc.sync.dma_start(x1a[:], xT_d[:, 0:4, TT0:TT0 + TT1])
                x_tiles[1] = (x1a, None)
            if o == 7 and len(tok_tiles) > 1:
                TT1 = tok_tiles[1]
                x1b = xpool.tile([128, 4 * TT1], f32r, tag="x0b")
                nc.sync.dma_start(x1b[:], xT_d[:, 4:8, TT0:TT0 + TT1])
                x_tiles[1] = (x_tiles[1][0], x1b)
            w2_strip = wpool.tile([128, 8 * 128], f32r, tag=f"w2_{o}")
            nc.sync.dma_start(w2_strip[:], w2_d[o])
            w2_sb[o] = w2_strip

        tile_pos = np.cumsum([0] + tok_tiles).tolist()
        pos = 0
        for t, TT in enumerate(tok_tiles):
            tsl = slice(pos, pos + TT)

            # prefetch x for tile t+1 (tiles 0 and 1 already issued)
            nt = t + 1
            if nt < len(tok_tiles) and x_tiles[nt] is None:
                x_pref = xpool.tile([128, 8 * tok_tiles[nt]], f32r, tag="x")
                nc.sync.dma_start(
                    x_pref[:],
                    xT_d[:, :, tile_pos[nt]:tile_pos[nt] + tok_tiles[nt]])
                x_tiles[nt] = x_pref

            x_sb = x_tiles[t]

            def xs(d):
                if isinstance(x_sb, tuple):
                    half = x_sb[d // 4]
                    return half[:, (d % 4) * TT:(d % 4 + 1) * TT]
                return x_sb[:, d * TT:(d + 1) * TT]

            # broadcast gate row across partitions: G[p, n] = g[n]
            g_ps = pgp.tile([128, TT], f32, tag="gps")
            nc.tensor.matmul(g_ps[:], ones_sb[:], g_sb[:, tsl],
                             start=True, stop=True)
            g_bc = gpool.tile([128, TT], f32, tag="gbc")
            nc.vector.tensor_copy(g_bc[:], g_ps[:])

            # layer 1: h^T[j,:] = relu(sum_d W1[d,j]^T x^T[d,:] + b1[j])
            h_sb = []
            for j in range(8):
                ph = php.tile([128, TT], f32, tag="ph")
                for d in range(8):
                    nc.tensor.matmul(ph[:],
                                     w1_sb[j][:, d * 128:(d + 1) * 128],
                                     xs(d),
                                     start=(d == 0), stop=(d == 7))
                ht = hpool.tile([128, TT], f32r, tag=f"h{j}")
                nc.vector.tensor_scalar(ht[:], ph[:], b1_sb[:, j:j + 1], 0.0,
                                        op0=ADD, op1=MAX)
                h_sb.append(ht)

            # layer 2 + gate: y^T[o,:] = (sum_j W2[j,o]^T h^T[j,:] + b2[o]) * g
            for o in range(8):
                py = pyp.tile([128, TT], f32, tag="py")
                for j in range(8):
                    nc.tensor.matmul(py[:],
                                     w2_sb[o][:, j * 128:(j + 1) * 128],
                                     h_sb[j][:],
                                     start=(j == 0), stop=(j == 7))
                yt = ypool.tile([128, TT], f32, tag=f"y{o}")
                nc.vector.scalar_tensor_tensor(yt[:], py[:], b2_sb[:, o:o + 1],
                                               g_bc[:], op0=ADD, op1=MULT)
                nc.sync.dma_start(yT_d[:, o, tsl], yt[:])

            pos += TT

    nc.compile()
    return nc


def _route(x, Wg, bg):
    """Host gating: fp32 softmax + top-2, matching jax.lax.top_k semantics."""
    logits = x @ Wg + bg
    m = logits.max(axis=1, keepdims=True)
    e = np.exp(logits - m)
    gates = e / e.sum(axis=1, keepdims=True)
    # stable argsort on negated values = ties broken by lower index (jax)
    order = np.argsort(-gates, axis=1, kind="stable")[:, :TOP_K]
    return gates, order


def _pack_w(W):
    """[1024, 1024] -> [8, 128, 8, 128]: strip s, part p, rowtile d, col r."""
    # out[s, p, d, r] = W[d*128+p, s*128+r]
    return np.ascontiguousarray(
        W.reshape(8, 128, 8, 128).transpose(2, 1, 0, 3))


def _make_in_maps(x, W1, b1, W2, b2, gates, order, tok_lists, C):
    in_maps = []
    for e in range(NUM_EXPERTS):
        toks = tok_lists[e]
        ne = len(toks)
        xT_e = np.zeros((128, 8, C), dtype=np.float32)
        # xT_e[p, d, :ne] = x[toks, d*128+p].T
        xT_e[:, :, :ne] = x[toks].T.reshape(8, 128, ne).transpose(1, 0, 2)
        g_e = np.zeros((1, C), dtype=np.float32)
        g_e[0, :ne] = gates[toks, e]
        in_maps.append({
            "xT": xT_e,
            "w1": _pack_w(W1[e]),
            "w2": _pack_w(W2[e]),
            "bb": np.ascontiguousarray(np.concatenate(
                [b1[e].reshape(8, 128).T, b2[e].reshape(8, 128).T], axis=1)),
            "go": np.concatenate(
                [g_e, np.ones((1, 128), dtype=np.float32)], axis=1),
        })
    return in_maps


def kernel(x, W1, b1, W2, b2, Wg, bg):
    from concourse import bass_utils

    x = np.ascontiguousarray(np.asarray(x, dtype=np.float32))
    W1 = np.asarray(W1, dtype=np.float32)
    b1 = np.asarray(b1, dtype=np.float32)
    W2 = np.asarray(W2, dtype=np.float32)
    b2 = np.asarray(b2, dtype=np.float32)
    Wg = np.asarray(Wg, dtype=np.float32)
    bg = np.asarray(bg, dtype=np.float32)
    n = x.shape[0]

    gates, order = _route(x, Wg, bg)
    tok_lists = [np.where((order == e).any(axis=1))[0] for e in range(NUM_EXPERTS)]
    max_load = max(len(t) for t in tok_lists)
    C, tok_tiles = _plan_tiles(max_load)

    key = (C, tuple(tok_tiles))
    if key not in _prog_cache:
        _prog_cache[key] = _build_program((C, tok_tiles))
    nc = _prog_cache[key]

    in_maps = _make_in_maps(x, W1, b1, W2, b2, gates, order, tok_lists, C)
    res = bass_utils.run_bass_kernel_spmd(nc, in_maps, list(range(NUM_EXPERTS)))
    # yT result: [128, 8, C] -> y_e[c, o*128+p] = yT[p, o, c]
    yT_all = np.stack([res.results[e]["yT"] for e in range(NUM_EXPERTS)])

    # scatter-add the two expert contributions per token (already gated)
    slot = np.zeros((NUM_EXPERTS, n), dtype=np.int64)
    for e in range(NUM_EXPERTS):
        slot[e, tok_lists[e]] = np.arange(len(tok_lists[e]))
    rows = np.arange(n)
    # gather columns: result [n, 128, 8] -> reshape to [n, 1024]
    out = np.zeros((n, D), dtype=np.float32)
    for k in range(TOP_K):
        ek = order[:, k]
        picked = yT_all[ek, :, :, slot[ek, rows]]   # [n, 128, 8]
        out += picked.transpose(0, 2, 1).reshape(n, D)
    return out



# revision 3
# speedup vs baseline: 1.2639x; 1.2639x over previous
"""Trainium2 Bass kernel for an 8-expert top-2 MoE layer.

Strategy (expert-parallel, per the sharding hint): the host computes the
tiny gating matmul + softmax + top-2 routing, gathers each expert's
assigned tokens, and ships one expert per NeuronCore. Each core runs the
2-layer MLP for its expert over its assigned tokens, applies the gate
weights on-device, and the host scatter-adds the two expert
contributions per token.

The heavy matmuls run in fp8 (e4m3) with perf_mode=DoubleRow: each
matmul consumes a K=256 contraction block as two 128-row slots packed
per PE cell, processing rows at 0.5 cycles each - 2x the bf16/f32r
rate per pass and 4x fewer passes than f32r's K=128 sweeps. Pure e4m3
(3 mantissa bits) is far too coarse for the 2e-2 gate, so every operand
is carried as an (hi, lo) pair of e4m3 tensors AT THE SAME SCALE:
hi = e4m3(v), lo = e4m3(v - hi). Because lo shares hi's scale it
accumulates into the same PSUM group with no extra eviction work, and
the subnormal flush it suffers is ~2^-17 relative - negligible. Each
layer then runs three DoubleRow passes (hi*Whi + lo*Whi + hi*Wlo),
giving ~1e-3 relative error at 6/16 the f32r PE cost.

Scales are pure powers of two folded host-side so no extra on-device
ops are needed: x is shipped as x*2^5 (max |x|*32 ~ 165 < 240, the TRN
e4m3 max), W as W*2^7, so layer-1 PSUM is h_pre*2^12. The ScalarE
activation op evicts h = relu(P*2^-7 + b1*2^5) (bias per-partition,
host-prescaled) straight to f32, a second ScalarE copy quantizes to
e4m3 (h*2^5), and one DVE op forms the residual h_lo = h*2^5 - h_hi.
Layer-2 PSUM is y*2^12; the eviction computes (P + b2*2^12) * (g*2^-12)
in one DVE op (bias and gate host-prescaled), emitting bf16.

Everything else follows the f32r baseline: the per-token gate row is
broadcast across partitions with one K=1 matmul, transfers are batched
per j-strip / per token tile, token tiles are 512 wide (one PSUM bank)
with a trimmed tail tile, and dummy matmuls run in the shadow of the
initial DMA ramp to engage the PE clock ramp before the real matmuls.
"""

import numpy as np
import ml_dtypes

NUM_EXPERTS = 8
TOP_K = 2
D = 1024

E4 = ml_dtypes.float8_e4m3  # TRN FP8_EXP4: max normal 240
SX = 2.0 ** 5    # x scale
SW = 2.0 ** 7    # weight scale
SH = 2.0 ** 5    # h scale (= SX * SW * 2^-7)

_prog_cache = {}


def _plan_tiles(max_load):
    """Token-tile sizes covering max_load: 512s plus a trimmed tail tile."""
    C = -(-max_load // 32) * 32  # mult of 32 keeps DR middle-dim steps %16==0
    tiles = [512] * (C // 512)
    if C % 512:
        tiles.append(C % 512)
    return C, tiles


def _build_program(tile_plan):
    """Per-core Bass program: one expert's fp8 DoubleRow MLP over C tokens."""
    from contextlib import ExitStack

    import concourse.tile as tile
    from concourse import bacc, mybir

    f32 = mybir.dt.float32
    f32r = mybir.dt.float32r
    f8 = mybir.dt.float8e4
    bf16 = mybir.dt.bfloat16
    ADD = mybir.AluOpType.add
    MAX = mybir.AluOpType.max
    MULT = mybir.AluOpType.mult
    RELU = mybir.ActivationFunctionType.Relu
    COPY = mybir.ActivationFunctionType.Copy
    DR = mybir.MatmulPerfMode.DoubleRow

    C, tok_tiles = tile_plan

    nc = bacc.Bacc("TRN2", target_bir_lowering=False, debug=False,
                   num_devices=NUM_EXPERTS)

    # host-packed layouts (see _make_in_maps):
    #   xh/xl: [128, 8, C]      xh[p, d, c] = e4m3(x_gathered[c, d*128+p]*2^5)
    #   w*:    [8, 128, 8, 128] w[j, p, d, m] = e4m3(W[d*128+p, j*128+m]*2^7)
    #   bb:    [128, 16]        [b1*2^5 | b2*2^12] per-partition columns
    #   go:    [1, C + 128]     [gate row * 2^-12 | ones row]
    #   yT:    [128, 8, C]      yT[p, o, c] = y[c, o*128+p] * gate[c]  (bf16)
    xh_d = nc.dram_tensor("xh", [128, 8, C], f8, kind="ExternalInput").ap()
    xl_d = nc.dram_tensor("xl", [128, 8, C], f8, kind="ExternalInput").ap()
    w1h_d = nc.dram_tensor("w1h", [8, 128, 8, 128], f8, kind="ExternalInput").ap()
    w1l_d = nc.dram_tensor("w1l", [8, 128, 8, 128], f8, kind="ExternalInput").ap()
    w2h_d = nc.dram_tensor("w2h", [8, 128, 8, 128], f8, kind="ExternalInput").ap()
    w2l_d = nc.dram_tensor("w2l", [8, 128, 8, 128], f8, kind="ExternalInput").ap()
    bb_d = nc.dram_tensor("bb", [128, 16], f32, kind="ExternalInput").ap()
    go_d = nc.dram_tensor("go", [1, C + 128], f32r, kind="ExternalInput").ap()
    yT_d = nc.dram_tensor("yT", [128, 8, C], bf16, kind="ExternalOutput").ap()

    with tile.TileContext(nc) as tc, ExitStack() as ctx:
        wpool = ctx.enter_context(tc.tile_pool(name="w", bufs=1))
        cpool = ctx.enter_context(tc.tile_pool(name="const", bufs=1))
        xpool = ctx.enter_context(tc.tile_pool(name="x", bufs=2))
        hfpool = ctx.enter_context(tc.tile_pool(name="hf", bufs=2))
        hpool = ctx.enter_context(tc.tile_pool(name="h", bufs=2))
        ypool = ctx.enter_context(tc.tile_pool(name="y", bufs=2))
        gpool = ctx.enter_context(tc.tile_pool(name="g", bufs=2))
        php = ctx.enter_context(tc.tile_pool(name="ph", bufs=3, space="PSUM"))
        pyp = ctx.enter_context(tc.tile_pool(name="py", bufs=3, space="PSUM"))
        pgp = ctx.enter_context(tc.tile_pool(name="pg", bufs=2, space="PSUM"))

        # tiny constants first (merged transfers)
        bb_sb = cpool.tile([128, 16], f32, tag="bb")
        nc.sync.dma_start(bb_sb[:], bb_d[:])
        go_sb = cpool.tile([1, C + 128], f32r, tag="go")
        nc.sync.dma_start(go_sb[:], go_d[:])
        g_sb = go_sb[:, 0:C]
        ones_sb = go_sb[:, C:C + 128]

        # PE warm-up in the shadow of the initial DMA ramp so the clock is
        # at max speed when the real matmuls arrive
        warm = pgp.tile([128, min(C, 512)], f32, tag="gps")
        for _ in range(6):
            nc.tensor.matmul(warm[:], ones_sb[:], g_sb[:, 0:min(C, 512)],
                             start=True, stop=True)

        # weight + x streams in consumption order
        TT0 = tok_tiles[0]
        w1h_sb = []
        for j in range(8):
            t = wpool.tile([128, 8, 128], f8, tag=f"w1h_{j}")
            nc.sync.dma_start(t[:], w1h_d[j])
            w1h_sb.append(t)
        x_tiles = [None] * len(tok_tiles)
        xh0 = xpool.tile([128, 8, TT0], f8, tag="xh")
        nc.sync.dma_start(xh0[:], xh_d[:, :, 0:TT0])
        xl0 = xpool.tile([128, 8, TT0], f8, tag="xl")
        nc.sync.dma_start(xl0[:], xl_d[:, :, 0:TT0])
        x_tiles[0] = (xh0, xl0)
        w1l_sb, w2h_sb, w2l_sb = [], [], []
        for j in range(8):
            t = wpool.tile([128, 8, 128], f8, tag=f"w1l_{j}")
            nc.sync.dma_start(t[:], w1l_d[j])
            w1l_sb.append(t)
        for o in range(8):
            t = wpool.tile([128, 8, 128], f8, tag=f"w2h_{o}")
            nc.sync.dma_start(t[:], w2h_d[o])
            w2h_sb.append(t)
        for o in range(8):
            t = wpool.tile([128, 8, 128], f8, tag=f"w2l_{o}")
            nc.sync.dma_start(t[:], w2l_d[o])
            w2l_sb.append(t)

        tile_pos = np.cumsum([0] + tok_tiles).tolist()
        for t, TT in enumerate(tok_tiles):
            t0 = tile_pos[t]
            tsl = slice(t0, t0 + TT)

            # prefetch x for tile t+1
            nt = t + 1
            if nt < len(tok_tiles) and x_tiles[nt] is None:
                NTT = tok_tiles[nt]
                nh = xpool.tile([128, 8, NTT], f8, tag="xh")
                nc.sync.dma_start(
                    nh[:], xh_d[:, :, tile_pos[nt]:tile_pos[nt] + NTT])
                nl = xpool.tile([128, 8, NTT], f8, tag="xl")
                nc.sync.dma_start(
                    nl[:], xl_d[:, :, tile_pos[nt]:tile_pos[nt] + NTT])
                x_tiles[nt] = (nh, nl)

            xh_sb, xl_sb = x_tiles[t]

            # broadcast gate row across partitions: G[p, n] = g[n] * 2^-12
            g_ps = pgp.tile([128, TT], f32, tag="gps")
            nc.tensor.matmul(g_ps[:], ones_sb[:], g_sb[:, tsl],
                             start=True, stop=True)
            g_bc = gpool.tile([128, TT], f32, tag="gbc")
            nc.vector.tensor_copy(g_bc[:], g_ps[:])

            # layer 1: three DoubleRow passes per j-strip into one PSUM group
            hf = hfpool.tile([128, 8, TT], f32, tag="hf")
            hh = hpool.tile([128, 8, TT], f8, tag="hh")
            hl = hpool.tile([128, 8, TT], f8, tag="hl")
            for j in range(8):
                ph = php.tile([128, TT], f32, tag="ph")
                for p in range(4):
                    nc.tensor.matmul(ph[:], w1h_sb[j][:, 2 * p:2 * p + 2, :],
                                     xh_sb[:, 2 * p:2 * p + 2, :],
                                     start=(p == 0), stop=False, perf_mode=DR)
                for p in range(4):
                    nc.tensor.matmul(ph[:], w1h_sb[j][:, 2 * p:2 * p + 2, :],
                                     xl_sb[:, 2 * p:2 * p + 2, :],
                                     start=False, stop=False, perf_mode=DR)
                for p in range(4):
                    nc.tensor.matmul(ph[:], w1l_sb[j][:, 2 * p:2 * p + 2, :],
                                     xh_sb[:, 2 * p:2 * p + 2, :],
                                     start=False, stop=(p == 3), perf_mode=DR)
                # h*2^5 = relu(P*2^-7 + b1*2^5); then split to e4m3 hi/lo
                nc.scalar.activation(hf[:, j, :], ph[:], RELU,
                                     bias=bb_sb[:, j:j + 1], scale=2.0 ** -7)
                nc.scalar.activation(hh[:, j, :], hf[:, j, :], COPY)
                nc.vector.scalar_tensor_tensor(hl[:, j, :], hh[:, j, :], -1.0,
                                               hf[:, j, :], op0=MULT, op1=ADD)

            # layer 2: three DoubleRow passes per o-strip; fused gate eviction
            yt = ypool.tile([128, 8, TT], bf16, tag="yt")
            for o in range(8):
                py = pyp.tile([128, TT], f32, tag="py")
                for q in range(4):
                    nc.tensor.matmul(py[:], w2h_sb[o][:, 2 * q:2 * q + 2, :],
                                     hh[:, 2 * q:2 * q + 2, :],
                                     start=(q == 0), stop=False, perf_mode=DR)
                for q in range(4):
                    nc.tensor.matmul(py[:], w2h_sb[o][:, 2 * q:2 * q + 2, :],
                                     hl[:, 2 * q:2 * q + 2, :],
                                     start=False, stop=False, perf_mode=DR)
                for q in range(4):
                    nc.tensor.matmul(py[:], w2l_sb[o][:, 2 * q:2 * q + 2, :],
                                     hh[:, 2 * q:2 * q + 2, :],
                                     start=False, stop=(q == 3), perf_mode=DR)
                # y*g = (P + b2*2^12) * (g*2^-12)
                nc.vector.scalar_tensor_tensor(yt[:, o, :], py[:],
                                               bb_sb[:, 8 + o:9 + o],
                                               g_bc[:], op0=ADD, op1=MULT)
            nc.sync.dma_start(yT_d[:, :, tsl], yt[:])

    nc.compile()
    return nc


def _route(x, Wg, bg):
    """Host gating: fp32 softmax + top-2, matching jax.lax.top_k semantics."""
    logits = x @ Wg + bg
    m = logits.max(axis=1, keepdims=True)
    e = np.exp(logits - m)
    gates = e / e.sum(axis=1, keepdims=True)
    order = np.argsort(-gates, axis=1, kind="stable")[:, :TOP_K]
    return gates, order


def _q8(a):
    return a.astype(E4)


def _split8(a, s):
    """v*s -> (hi, lo) e4m3 pair at the same scale: hi+lo ~= v*s."""
    vs = a * np.float32(s)
    hi = vs.astype(E4)
    lo = (vs - hi.astype(np.float32)).astype(E4)
    return hi, lo


def _pack_w(W8):
    """[1024, 1024] e4m3 -> [8, 128, 8, 128]: strip j, part p, kblock d, m."""
    # out[j, p, d, m] = W[d*128+p, j*128+m]
    return np.ascontiguousarray(
        W8.reshape(8, 128, 8, 128).transpose(2, 1, 0, 3))


def _pack_xT(x8, toks, C):
    """tokens' rows of x8 [N,1024] e4m3 -> [128, 8, C] (p, d, c) layout."""
    ne = len(toks)
    out = np.zeros((128, 8, C), dtype=E4)
    out[:, :, :ne] = x8[toks].T.reshape(8, 128, ne).transpose(1, 0, 2)
    return out


def _make_in_maps(x, W1, b1, W2, b2, gates, order, tok_lists, C):
    xh8, xl8 = _split8(x, SX)
    in_maps = []
    for e in range(NUM_EXPERTS):
        toks = tok_lists[e]
        ne = len(toks)
        w1h, w1l = _split8(W1[e], SW)
        w2h, w2l = _split8(W2[e], SW)
        g_e = np.zeros((1, C), dtype=np.float32)
        g_e[0, :ne] = gates[toks, e] * np.float32(1.0 / (SH * SW))
        bb = np.concatenate(
            [b1[e].reshape(8, 128).T * np.float32(SH),
             b2[e].reshape(8, 128).T * np.float32(SH * SW)],
            axis=1)
        in_maps.append({
            "xh": _pack_xT(xh8, toks, C),
            "xl": _pack_xT(xl8, toks, C),
            "w1h": _pack_w(w1h),
            "w1l": _pack_w(w1l),
            "w2h": _pack_w(w2h),
            "w2l": _pack_w(w2l),
            "bb": np.ascontiguousarray(bb),
            "go": np.concatenate(
                [g_e, np.ones((1, 128), dtype=np.float32)], axis=1),
        })
    return in_maps


def kernel(x, W1, b1, W2, b2, Wg, bg):
    from concourse import bass_utils

    x = np.ascontiguousarray(np.asarray(x, dtype=np.float32))
    W1 = np.asarray(W1, dtype=np.float32)
    b1 = np.asarray(b1, dtype=np.float32)
    W2 = np.asarray(W2, dtype=np.float32)
    b2 = np.asarray(b2, dtype=np.float32)
    Wg = np.asarray(Wg, dtype=np.float32)
    bg = np.asarray(bg, dtype=np.float32)
    n = x.shape[0]

    gates, order = _route(x, Wg, bg)
    tok_lists = [np.where((order == e).any(axis=1))[0]
                 for e in range(NUM_EXPERTS)]
    max_load = max(len(t) for t in tok_lists)
    C, tok_tiles = _plan_tiles(max_load)

    key = (C, tuple(tok_tiles))
    if key not in _prog_cache:
        _prog_cache[key] = _build_program((C, tok_tiles))
    nc = _prog_cache[key]

    in_maps = _make_in_maps(x, W1, b1, W2, b2, gates, order, tok_lists, C)
    res = bass_utils.run_bass_kernel_spmd(nc, in_maps, list(range(NUM_EXPERTS)))
    # yT result: [128, 8, C] bf16 -> y_e[c, o*128+p] = yT[p, o, c]
    yT_all = np.stack([np.asarray(res.results[e]["yT"]).astype(np.float32)
                       for e in range(NUM_EXPERTS)])

    # scatter-add the two expert contributions per token (already gated)
    slot = np.zeros((NUM_EXPERTS, n), dtype=np.int64)
    for e in range(NUM_EXPERTS):
        slot[e, tok_lists[e]] = np.arange(len(tok_lists[e]))
    rows = np.arange(n)
    out = np.zeros((n, D), dtype=np.float32)
    for k in range(TOP_K):
        ek = order[:, k]
        picked = yT_all[ek, :, :, slot[ek, rows]]   # [n, 128, 8]
        out += picked.transpose(0, 2, 1).reshape(n, D)
    return out
